# revision 4
# baseline (speedup 1.0000x reference)
import os
import numpy as np
import ml_dtypes
from contextlib import ExitStack, nullcontext

import concourse.bass as bass
import concourse.tile as tile
import concourse.bacc as bacc
import concourse.mybir as mybir
from concourse.bass_utils import run_bass_kernel_spmd

B, N, C, NS, S = 8, 4096, 128, 16, 8
CS = C // S          # 16
NT = N // 128        # 32 i-tiles
TBL = 384            # bf16 elems per table row: xk 128 | xv 128 | a 3 | pad -> 768B
BF16 = mybir.dt.bfloat16
F32 = mybir.dt.float32
I16 = mybir.dt.int16
AF = mybir.ActivationFunctionType
ALU = mybir.AluOpType
AX = mybir.AxisListType

_CACHE = {}


def _build_nc():
    nc = bacc.Bacc("TRN2", target_bir_lowering=False, debug=False)
    d = {}
    d["tf"] = nc.dram_tensor("tf", [C, N], F32, kind="ExternalInput")
    d["p3"] = nc.dram_tensor("p3", [3, N], F32, kind="ExternalInput")
    d["iw"] = nc.dram_tensor("iw", [128, N], I16, kind="ExternalInput")
    d["lin1w"] = nc.dram_tensor("lin1w", [C, C], F32, kind="ExternalInput")
    d["lp1w"] = nc.dram_tensor("lp1w", [3, 3], F32, kind="ExternalInput")
    for nm, sh in [("wqb", [C, C]), ("wkb", [C, C]), ("wvb", [C, C]),
                   ("lp2wb", [3, C]), ("lw1wb", [C, CS]), ("lw2wb", [CS, CS]),
                   ("lin3wb", [C, C]), ("m1wb", [C, 64]), ("m2wb", [64, 3]),
                   ("ident", [128, 128])]:
        d[nm] = nc.dram_tensor(nm, sh, BF16, kind="ExternalInput")
    for nm, p in [("bias1", C), ("bq", C), ("bk", C), ("bv", C), ("b3", 3),
                  ("lp2b", C), ("lwb1b", C), ("w1be", CS), ("lw2b", CS),
                  ("bn2b", C), ("bn3b", C), ("m1be", 64)]:
        d[nm] = nc.dram_tensor(nm, [p, 1], F32, kind="ExternalInput")
    tbl_d = nc.dram_tensor("tbl", [N, TBL], BF16, kind="Internal")
    out_d = nc.dram_tensor("out", [3, N], F32, kind="ExternalOutput")

    with tile.TileContext(nc) as tc:
        with ExitStack() as ctx:
            # ---- persistent SBUF tiles (one pool, unique tags) ----
            pers = ctx.enter_context(tc.tile_pool(name="pers", bufs=1))

            def ptile(shape, dtype, nm):
                return pers.tile(shape, dtype, name=nm, tag=nm)

            tf_sb = ptile([C, N], F32, "tf_sb")
            p3_sb = ptile([3, N], F32, "p3_sb")
            iw_sb = ptile([128, N], I16, "iw_sb")
            Xb = ptile([C, N], BF16, "Xb")
            xqb = ptile([C, N], BF16, "xqb")
            xkb = ptile([C, N], BF16, "xkb")
            xvb = ptile([C, N], BF16, "xvb")
            ab = ptile([3, N], BF16, "ab")
            y2b = ptile([C, N], BF16, "y2b")
            zb = ptile([C, N], BF16, "zb")
            h2b = ptile([64, N], BF16, "h2b")
            w_sb = {}
            for nm in ["lin1w", "lp1w", "wqb", "wkb", "wvb", "lp2wb", "lw1wb",
                       "lw2wb", "lin3wb", "m1wb", "m2wb", "ident", "bias1",
                       "bq", "bk", "bv", "b3", "lp2b", "lwb1b", "w1be",
                       "lw2b", "bn2b", "bn3b", "m1be"]:
                t = ptile(list(d[nm].shape), d[nm].dtype, nm + "_s")
                w_sb[nm] = t

            ps = ctx.enter_context(tc.tile_pool(name="ps", bufs=4, space=bass.MemorySpace.PSUM))
            tbp = ctx.enter_context(tc.tile_pool(name="tbp", bufs=2))
            gp = ctx.enter_context(tc.tile_pool(name="gp", bufs=1))
            hp = ctx.enter_context(tc.tile_pool(name="hp", bufs=2))
            prp = ctx.enter_context(tc.tile_pool(name="prp", bufs=1))
            tmp = ctx.enter_context(tc.tile_pool(name="tmp", bufs=2))
            wrp = ctx.enter_context(tc.tile_pool(name="wrp", bufs=1))
            w1p = ctx.enter_context(tc.tile_pool(name="w1p", bufs=1))
            ep = ctx.enter_context(tc.tile_pool(name="ep", bufs=1))
            erp = ctx.enter_context(tc.tile_pool(name="erp", bufs=1))
            vp = ctx.enter_context(tc.tile_pool(name="vp", bufs=1))
            vwp = ctx.enter_context(tc.tile_pool(name="vwp", bufs=1))
            sp = ctx.enter_context(tc.tile_pool(name="sp", bufs=2))
            op = ctx.enter_context(tc.tile_pool(name="op", bufs=2))

            def mm(out, lhsT, rhs):
                nc.tensor.matmul(out, lhsT, rhs, start=True, stop=True)

            KREP = int(os.environ.get("KREP", "1"))
            with (tc.For_i(0, KREP, name="rep") if KREP > 1 else nullcontext()):
                for nm in w_sb:
                    nc.gpsimd.dma_start(w_sb[nm][:], d[nm].ap())
                nc.gpsimd.dma_start(tf_sb[:], d["tf"].ap())
                nc.gpsimd.dma_start(p3_sb[:], d["p3"].ap())
                nc.gpsimd.dma_start(iw_sb[:], d["iw"].ap())
                # ---- phase A: projections ----
                for c0 in range(0, N, 512):
                    sl = bass.ts(c0 // 512, 512)
                    pt = ps.tile([128, 512], F32, name="psA", tag="ps")
                    mm(pt[:], w_sb["lin1w"][:], tf_sb[:, sl])
                    nc.scalar.activation(Xb[:, sl], pt[:], AF.Relu, bias=w_sb["bias1"][:])
                for c0 in range(0, N, 512):
                    sl = bass.ts(c0 // 512, 512)
                    for wname, bname, dst in [("wqb", "bq", xqb), ("wkb", "bk", xkb),
                                              ("wvb", "bv", xvb)]:
                        pt = ps.tile([128, 512], F32, name="psq", tag="ps")
                        mm(pt[:], w_sb[wname][:], Xb[:, sl])
                        nc.scalar.activation(dst[:, sl], pt[:], AF.Identity, bias=w_sb[bname][:])
                    pa = ps.tile([128, 512], F32, name="psa", tag="ps")
                    mm(pa[0:3, :], w_sb["lp1w"][:], p3_sb[:, sl])
                    nc.scalar.activation(ab[:, sl], pa[0:3, :], AF.Copy)

                # ---- phase B: build gather table in DRAM (point-major bf16 rows) ----
                for it in range(NT):
                    sl = bass.ts(it, 128)
                    row = tbp.tile([128, TBL], BF16, name="row")
                    ptk = ps.tile([128, 512], F32, name="ptk", tag="ps")
                    mm(ptk[:, 0:128], xkb[:, sl], w_sb["ident"][:])
                    nc.scalar.activation(row[:, 0:128], ptk[:, 0:128], AF.Copy)
                    ptv = ps.tile([128, 512], F32, name="ptv", tag="ps")
                    mm(ptv[:, 0:128], xvb[:, sl], w_sb["ident"][:])
                    nc.scalar.activation(row[:, 128:256], ptv[:, 0:128], AF.Copy)
                    pta = ps.tile([128, 512], F32, name="pta", tag="ps")
                    mm(pta[:, 0:3], ab[:, sl], w_sb["ident"][0:3, 0:3])
                    nc.scalar.activation(row[:, 256:259], pta[:, 0:3], AF.Copy)
                    nc.gpsimd.dma_start(tbl_d.ap()[it * 128:(it + 1) * 128, :], row[:])

                KPH = os.environ.get("KPHASE", "full")
                NT_C = 0 if KPH == "ab" else (1 if KPH == "c1" else NT)
                if KPH in ("ab", "c1"):
                    nc.gpsimd.dma_start(out_d.ap(), p3_sb[:])
                # ---- phase C: per-tile attention ----
                for it in range(NT_C):
                    sl = bass.ts(it, 128)
                    g = gp.tile([128, 4, 3, 512], BF16, name="g")
                    for c in range(4):
                        nc.gpsimd.dma_gather(g[:, c], tbl_d.ap(),
                                             iw_sb[:, it * 128 + c * 32:it * 128 + (c + 1) * 32],
                                             512, 512, TBL, transpose=True)
                    # h = relu(a_j - a_i + b3)
                    hf = hp.tile([3, 2048], BF16, name="hf", tag="h")
                    for c in range(4):
                        cs = bass.ts(c, 512)
                        aiv = ab[0:3, it * 128 + c * 32:it * 128 + (c + 1) * 32] \
                            .unsqueeze(2).broadcast_to((3, 32, NS))
                        nc.vector.scalar_tensor_tensor(
                            hf[:, cs].rearrange("p (n t) -> p n t", t=NS),
                            g[0:3, c, 2, :].rearrange("p (n t) -> p n t", t=NS),
                            0.0, aiv, ALU.bypass, ALU.subtract)
                    hb = hp.tile([3, 2048], BF16, name="hb", tag="h")
                    nc.scalar.activation(hb[:], hf[:], AF.Relu, bias=w_sb["b3"][:])
                    # p_r = lp2w.T @ h + lp2b
                    pr = prp.tile([128, 2048], BF16, name="pr")
                    for q in range(4):
                        qs = bass.ts(q, 512)
                        pp = ps.tile([128, 512], F32, name="ppr", tag="ps")
                        mm(pp[:], w_sb["lp2wb"][:], hb[:, qs])
                        nc.scalar.activation(pr[:, qs], pp[:], AF.Identity, bias=w_sb["lp2b"][:])
                    # w pre-act: xkg - xq + p_r
                    t1 = tmp.tile([128, 2048], BF16, name="t1", tag="t")
                    for c in range(4):
                        cs = bass.ts(c, 512)
                        xqv = xqb[:, it * 128 + c * 32:it * 128 + (c + 1) * 32] \
                            .unsqueeze(2).broadcast_to((128, 32, NS))
                        nc.vector.scalar_tensor_tensor(
                            t1[:, cs].rearrange("p (n t) -> p n t", t=NS),
                            g[:, c, 0, :].rearrange("p (n t) -> p n t", t=NS),
                            0.0, xqv, ALU.bypass, ALU.subtract)
                    t2 = tmp.tile([128, 2048], BF16, name="t2", tag="t")
                    nc.vector.scalar_tensor_tensor(t2[:], t1[:], 0.0, pr[:],
                                                   ALU.bypass, ALU.add)
                    wrel = wrp.tile([128, 2048], BF16, name="wrel")
                    nc.scalar.activation(wrel[:], t2[:], AF.Relu, bias=w_sb["lwb1b"][:])
                    # w1 + relu, w2 + exp
                    w1r = w1p.tile([CS, 2048], BF16, name="w1r")
                    for q in range(4):
                        qs = bass.ts(q, 512)
                        pw = ps.tile([128, 512], F32, name="pw1", tag="ps")
                        mm(pw[0:CS, :], w_sb["lw1wb"][:], wrel[:, qs])
                        nc.scalar.activation(w1r[:, qs], pw[0:CS, :], AF.Relu,
                                             bias=w_sb["w1be"][:])
                    E = ep.tile([CS, 2048], F32, name="E")
                    for q in range(4):
                        qs = bass.ts(q, 512)
                        pw = ps.tile([128, 512], F32, name="pw2", tag="ps")
                        mm(pw[0:CS, :], w_sb["lw2wb"][:], w1r[:, qs])
                        nc.scalar.activation(E[:, qs], pw[0:CS, :], AF.Exp,
                                             bias=w_sb["lw2b"][:])
                    # softmax denom + replicate
                    Z = sp.tile([CS, 128], F32, name="Z")
                    nc.vector.tensor_reduce(Z[:], E[:].rearrange("p (n t) -> p n t", t=NS),
                                            AX.X, ALU.add)
                    R = sp.tile([CS, 128], F32, name="R")
                    nc.vector.reciprocal(R[:], Z[:])
                    Erep = erp.tile([128, 2048], F32, name="Erep")
                    Rrep = sp.tile([128, 128], F32, name="Rrep")
                    for r in range(8):
                        nc.gpsimd.dma_start(Erep[16 * r:16 * (r + 1), :], E[:])
                        nc.gpsimd.dma_start(Rrep[16 * r:16 * (r + 1), :], R[:])
                    # V = xvg + p_r ; VW = V * Erep ; y = sum_t VW * R
                    V = vp.tile([128, 2048], BF16, name="V")
                    for c in range(4):
                        cs = bass.ts(c, 512)
                        nc.vector.scalar_tensor_tensor(V[:, cs], g[:, c, 1, :],
                                                       0.0, pr[:, cs],
                                                       ALU.bypass, ALU.add)
                    VW = vwp.tile([128, 2048], F32, name="VW")
                    nc.vector.scalar_tensor_tensor(VW[:], V[:], 0.0, Erep[:],
                                                   ALU.bypass, ALU.mult)
                    yt = sp.tile([128, 128], F32, name="yt")
                    nc.vector.tensor_reduce(yt[:], VW[:].rearrange("p (n t) -> p n t", t=NS),
                                            AX.X, ALU.add)
                    yn = sp.tile([128, 128], F32, name="yn")
                    nc.vector.scalar_tensor_tensor(yn[:], yt[:], 0.0, Rrep[:],
                                                   ALU.bypass, ALU.mult)
                    nc.scalar.activation(y2b[:, sl], yn[:], AF.Relu, bias=w_sb["bn2b"][:])

                # ---- phase D: epilogue ----
                for c0 in (range(0, N, 512) if KPH == "full" else []):
                    sl = bass.ts(c0 // 512, 512)
                    pl = ps.tile([128, 512], F32, name="pl3", tag="ps")
                    mm(pl[:], w_sb["lin3wb"][:], y2b[:, sl])
                    zf = op.tile([128, 512], F32, name="zf", tag="o")
                    nc.vector.scalar_tensor_tensor(zf[:], pl[:], w_sb["bn3b"][:],
                                                   tf_sb[:, sl], ALU.add, ALU.add)
                    nc.scalar.activation(zb[:, sl], zf[:], AF.Relu)
                for c0 in (range(0, N, 512) if KPH == "full" else []):
                    sl = bass.ts(c0 // 512, 512)
                    pm = ps.tile([128, 512], F32, name="pm1", tag="ps")
                    mm(pm[0:64, :], w_sb["m1wb"][:], zb[:, sl])
                    nc.scalar.activation(h2b[:, sl], pm[0:64, :], AF.Relu,
                                         bias=w_sb["m1be"][:])
                for c0 in (range(0, N, 512) if KPH == "full" else []):
                    sl = bass.ts(c0 // 512, 512)
                    pm = ps.tile([128, 512], F32, name="pm2", tag="ps")
                    mm(pm[0:3, :], w_sb["m2wb"][:], h2b[:, sl])
                    ob = op.tile([3, 512], F32, name="ob", tag="o")
                    nc.vector.scalar_tensor_tensor(ob[:], pm[0:3, :], 0.0,
                                                   p3_sb[:, sl], ALU.bypass, ALU.add)
                    nc.gpsimd.dma_start(out_d.ap()[:, sl], ob[:])

    nc.compile()
    return nc


def kernel(**inputs):
    f32 = lambda k: np.asarray(inputs[k], np.float32)
    pxo = f32("pxo")                       # [B,N,3]
    tf = f32("transf_features")            # [B,C,N]
    bf = lambda a: np.ascontiguousarray(a).astype(ml_dtypes.bfloat16)
    col = lambda k: np.ascontiguousarray(f32(k).reshape(-1, 1))

    shared = {
        "lin1w": np.ascontiguousarray(f32("lin1w")),
        "lp1w": np.ascontiguousarray(f32("lp1w")),
        "wqb": bf(f32("wq")), "wkb": bf(f32("wk")), "wvb": bf(f32("wv")),
        "lp2wb": bf(f32("lp2w")), "lw1wb": bf(f32("lw1w")),
        "lw2wb": bf(f32("lw2w")), "lin3wb": bf(f32("lin3w")),
        "m1wb": bf(f32("m1w")), "m2wb": bf(f32("m2w")),
        "ident": bf(np.eye(128, dtype=np.float32)),
        "bias1": col("bn1b"), "bq": col("bq"), "bk": col("bk"), "bv": col("bv"),
        "b3": np.ascontiguousarray((f32("lp1b") + f32("lpbb")).reshape(-1, 1)),
        "lp2b": col("lp2b"), "lwb1b": col("lwb1b"),
        "w1be": np.ascontiguousarray((f32("lw1b") + f32("lwb2b")).reshape(-1, 1)),
        "lw2b": col("lw2b"), "bn2b": col("bn2b"), "bn3b": col("bn3b"),
        "m1be": np.ascontiguousarray((f32("m1b") + f32("mbb")).reshape(-1, 1)),
    }

    in_maps = []
    for b in range(B):
        p = pxo[b]                                        # [N,3]
        sq = (p * p).sum(1)
        dmat = sq[:, None] + sq[None, :] - 2.0 * (p @ p.T)
        idx = np.argpartition(dmat, NS, axis=1)[:, :NS]   # [N,16] smallest set
        iw = np.empty((128, N), np.int16)
        for it in range(NT):
            L = idx[it * 128:(it + 1) * 128, :].reshape(2048)
            blk = L.reshape(128, 16).T.astype(np.int16)   # [16,128] wrapped
            iw[:, it * 128:(it + 1) * 128] = np.tile(blk, (8, 1))
        m = dict(shared)
        m["tf"] = np.ascontiguousarray(tf[b])
        m["p3"] = np.ascontiguousarray(p.T)
        m["iw"] = iw
        in_maps.append(m)

    _CACHE["in_maps"] = in_maps
    if "nc" not in _CACHE:
        _CACHE["nc"] = _build_nc()
    res = run_bass_kernel_spmd(_CACHE["nc"], in_maps, core_ids=list(range(8)))
    return np.stack([np.asarray(res.results[i]["out"], np.float32)
                     for i in range(B)], axis=0)



# revision 65
# speedup vs baseline: 2.5522x; 2.5522x over previous
import os
import numpy as np
import ml_dtypes
from contextlib import ExitStack, nullcontext

import concourse.bass as bass
import concourse.tile as tile
import concourse.bacc as bacc
import concourse.mybir as mybir
from concourse.bass_utils import run_bass_kernel_spmd

B, N, C, NS, S = 8, 4096, 128, 16, 8
CS = C // S          # 16
NT = N // 128        # 32 i-tiles
TBL = 384            # bf16 elems per table row: xk 128 | xv 128 | a 3 | zeros -> 768B
BF16 = mybir.dt.bfloat16
F32 = mybir.dt.float32
I16 = mybir.dt.int16
AF = mybir.ActivationFunctionType
ALU = mybir.AluOpType
AX = mybir.AxisListType

_CACHE = {}


def _build_nc():
    nc = bacc.Bacc("TRN2", target_bir_lowering=False, debug=False,
                   num_swdge_queues=2)
    d = {}
    d["iw"] = nc.dram_tensor("iw", [128, N], I16, kind="ExternalInput")
    for nm, sh in [("tfb", [C, N]), ("p3b", [3, N]),
                   ("lin1wb", [C, C]), ("lp1wb", [3, 16]),
                   ("wqb", [C, C]), ("wkb", [C, C]), ("wvb", [C, C]),
                   ("lp2wb", [3, C]), ("lw1wb", [C, CS]), ("lw2repb", [CS, C]),
                   ("lin3wb", [C, C]), ("m1wb", [C, 64]), ("m2wb", [64, 3]),
                   ("ident", [128, 128])]:
        d[nm] = nc.dram_tensor(nm, sh, BF16, kind="ExternalInput")
    for nm, p in [("bias1", C), ("qmbias", C), ("bk", C), ("bv", C), ("b3", 3),
                  ("lp2b", C), ("w1be", CS), ("lw2brep", C),
                  ("bn2b", C), ("bn3b", C), ("m1be", 64)]:
        d[nm] = nc.dram_tensor(nm, [p, 1], F32, kind="ExternalInput")
    tbl_d = nc.dram_tensor("tbl", [N, TBL], BF16, kind="Internal")
    out_d = nc.dram_tensor("out", [3, N], F32, kind="ExternalOutput")

    with tile.TileContext(nc) as tc:
        with ExitStack() as ctx:
            # ---- persistent SBUF tiles ----
            pers = ctx.enter_context(tc.tile_pool(name="pers", bufs=1))

            def ptile(shape, dtype, nm):
                return pers.tile(shape, dtype, name=nm, tag=nm)

            tfb_sb = ptile([C, N], BF16, "tfb_sb")
            p3b_sb = ptile([3, N], BF16, "p3b_sb")
            iw_sb = ptile([128, N], I16, "iw_sb")
            Xb = ptile([C, N], BF16, "Xb")
            qmb = ptile([C, N], BF16, "qmb")       # lwb1b - bq - X@wq
            xkb = ptile([C, N], BF16, "xkb")
            xvb = ptile([C, N], BF16, "xvb")
            ab = ptile([16, N], BF16, "ab")        # raw a = lp1w.T @ p (rows 3..15 zero)
            y2b = ptile([C, N], BF16, "y2b")
            zeroC = ptile([C, 512], BF16, "zeroC")
            w_sb = {}
            for nm in ["lin1wb", "lp1wb", "wqb", "wkb", "wvb",
                       "lp2wb", "lw1wb", "lw2repb", "lin3wb", "m1wb", "m2wb",
                       "ident", "bias1", "qmbias", "bk", "bv", "b3", "lp2b",
                       "w1be", "lw2brep", "bn2b", "bn3b", "m1be"]:
                t = ptile(list(d[nm].shape), d[nm].dtype, nm + "_s")
                w_sb[nm] = t

            ps1 = ctx.enter_context(tc.tile_pool(name="ps1", bufs=3, space=bass.MemorySpace.PSUM))
            ps2 = ctx.enter_context(tc.tile_pool(name="ps2", bufs=3, space=bass.MemorySpace.PSUM))
            ps3 = ctx.enter_context(tc.tile_pool(name="ps3", bufs=2, space=bass.MemorySpace.PSUM))
            tbp = ctx.enter_context(tc.tile_pool(name="tbp", bufs=2))
            gp = ctx.enter_context(tc.tile_pool(name="gp", bufs=2))
            hp = ctx.enter_context(tc.tile_pool(name="hp", bufs=3))
            prp = ctx.enter_context(tc.tile_pool(name="prp", bufs=4))
            wqp = ctx.enter_context(tc.tile_pool(name="wqp", bufs=4))
            wpp = ctx.enter_context(tc.tile_pool(name="wpp", bufs=4))
            wrp = ctx.enter_context(tc.tile_pool(name="wrp", bufs=4))
            w1p = ctx.enter_context(tc.tile_pool(name="w1p", bufs=4))
            vp = ctx.enter_context(tc.tile_pool(name="vp", bufs=4))
            ewp = ctx.enter_context(tc.tile_pool(name="ewp", bufs=2))
            fp = ctx.enter_context(tc.tile_pool(name="fp", bufs=3))
            sp = ctx.enter_context(tc.tile_pool(name="sp", bufs=3))
            op = ctx.enter_context(tc.tile_pool(name="op", bufs=3))
            zp = ctx.enter_context(tc.tile_pool(name="zp", bufs=3))

            def mm(out, lhsT, rhs, start=True, stop=True):
                nc.tensor.matmul(out, lhsT, rhs, start=start, stop=stop)

            KREP = int(os.environ.get("KREP", "1"))
            KPH = os.environ.get("KPHASE", "full")
            with (tc.For_i(0, KREP, name="rep") if KREP > 1 else nullcontext()):
                for nm in w_sb:
                    nc.sync.dma_start(w_sb[nm][:], d[nm].ap())
                nc.sync.dma_start(tfb_sb[:], d["tfb"].ap())
                nc.sync.dma_start(p3b_sb[:], d["p3b"].ap())
                nc.sync.dma_start(iw_sb[:], d["iw"].ap())
                nc.vector.memset(zeroC[:], 0.0)

                # ---- phase A: projections ----
                for c0 in range(0, N, 512):
                    sl = bass.ts(c0 // 512, 512)
                    pt = ps1.tile([128, 512], F32, name="psA", tag="psA")
                    mm(pt[:], w_sb["lin1wb"][:], tfb_sb[:, sl])
                    nc.scalar.activation(Xb[:, sl], pt[:], AF.Relu, bias=w_sb["bias1"][:])
                for c0 in range(0, N, 512):
                    sl = bass.ts(c0 // 512, 512)
                    pq = ps2.tile([128, 512], F32, name="psq", tag="psq")
                    mm(pq[:], w_sb["wqb"][:], Xb[:, sl])
                    nc.scalar.activation(qmb[:, sl], pq[:], AF.Identity,
                                         bias=w_sb["qmbias"][:], scale=-1.0)
                    pk = ps3.tile([128, 512], F32, name="psk", tag="psk")
                    mm(pk[:], w_sb["wkb"][:], Xb[:, sl])
                    nc.vector.scalar_tensor_tensor(xkb[:, sl], pk[:], w_sb["bk"][:],
                                                   zeroC[:, 0:512], ALU.add, ALU.add)
                    pv = ps1.tile([128, 512], F32, name="psv", tag="psA")
                    mm(pv[:], w_sb["wvb"][:], Xb[:, sl])
                    nc.vector.scalar_tensor_tensor(xvb[:, sl], pv[:], w_sb["bv"][:],
                                                   zeroC[:, 0:512], ALU.add, ALU.add)
                    pa = ps2.tile([128, 512], F32, name="psa", tag="psq")
                    mm(pa[0:16, :], w_sb["lp1wb"][:], p3b_sb[:, sl])
                    nc.scalar.activation(ab[:, sl], pa[0:16, :], AF.Copy)

                # ---- phase B: build gather table via DMA transpose ----
                for it in range(NT):
                    sl = bass.ts(it, 128)
                    row = tbp.tile([128, TBL], BF16, name="row")
                    ptx = ps3.tile([128, 512], F32, name="ptx", tag="psk")
                    mm(ptx[:, 0:128], xkb[:, sl], w_sb["ident"][:])
                    mm(ptx[:, 128:256], xvb[:, sl], w_sb["ident"][:])
                    mm(ptx[:, 256:384], ab[:, sl], w_sb["ident"][0:16, :])
                    if it % 2 == 0:
                        nc.scalar.activation(row[:], ptx[:, 0:384], AF.Copy)
                    else:
                        nc.vector.scalar_tensor_tensor(row[:], ptx[:, 0:384], 0.0,
                                                       zeroC[:, 0:384], ALU.bypass,
                                                       ALU.add)
                    nc.sync.dma_start(tbl_d.ap()[it * 128:(it + 1) * 128, :], row[:])

                NT_C = {"ab": 0, "c1": 1, "c8": 8, "c16": 16}.get(KPH, NT)
                if KPH != "full":
                    oz = op.tile([3, 512], F32, name="oz", tag="ob")
                    nc.vector.memset(oz[:], 0.0)
                    nc.sync.dma_start(out_d.ap()[:, 0:512], oz[:])

                # ---- phase C: per-tile attention ----
                KGN = int(os.environ.get("KGN", "4"))      # gathers per tile
                HFENG = (nc.gpsimd if os.environ.get("KHF", "dve") == "pool"
                         else nc.vector)
                KWQ = os.environ.get("KWQ", "mix")         # pool | dve | mix
                ni = 2048 // KGN
                npc = 128 // KGN                           # points per gather
                for it in range(NT_C):
                    slt = bass.ts(it, 128)
                    g2 = gp.tile([128, KGN, 3, ni], BF16, name="g2")

                    def gv(j, q, pz=128):
                        # [pz, 512] view of row-chunk j for col-chunk q
                        c0 = q * 512
                        gi, lo = c0 // ni, c0 % ni
                        return g2[0:pz, gi, j, lo:lo + 512]

                    for gi in range(KGN):
                        nc.gpsimd.dma_gather(
                            g2[:, gi], tbl_d.ap(),
                            iw_sb[:, it * 128 + gi * (ni // 16):
                                  it * 128 + (gi + 1) * (ni // 16)],
                            ni, ni, TBL, transpose=True, queue_num=gi % 2)
                    # pair columns are t-major: col k = (t, n) = (k//128, k%128)
                    # broadcasts over t keep innermost n stride-1 => DVE 2x legal
                    VW3 = ewp.tile([128, NS, 128], BF16, name="VW3", tag="VW3")
                    ER3 = ewp.tile([128, NS, 128], BF16, name="ER3", tag="ER3")
                    hf = hp.tile([3, 2048], BF16, name="hf", tag="hf")
                    HFENG.tensor_tensor(
                        hf[:].rearrange("p (c t n) -> p c t n", c=KGN, n=128),
                        g2[0:3, :, 2, :].rearrange("p c (t n) -> p c t n", n=128),
                        ab[0:3, slt].unsqueeze(1).unsqueeze(2)
                            .broadcast_to((3, KGN, NS // KGN, 128)),
                        ALU.subtract)
                    hb = hp.tile([3, 2048], BF16, name="hb", tag="hb")
                    nc.vector.tensor_scalar(hb[:], hf[:], w_sb["b3"][:], 0.0,
                                            ALU.add, ALU.max)
                    qmv = (qmb[:, slt].unsqueeze(1).broadcast_to((128, 4, 128)))
                    for q in range(4):
                        qs = bass.ts(q, 512)
                        qt = slice(q * 4, (q + 1) * 4)
                        pP = ps1.tile([128, 512], F32, name="pP", tag="psA")
                        mm(pP[:], w_sb["lp2wb"][:], hb[:, qs])
                        pr = prp.tile([128, 512], BF16, name="pr")
                        nc.scalar.activation(pr[:], pP[:], AF.Identity,
                                             bias=w_sb["lp2b"][:])
                        # wq1 = pr + (qmb_i bcast over t)  [2x]
                        wq1 = wqp.tile([128, 512], BF16, name="wq1")
                        nc.vector.tensor_tensor(
                            wq1[:].rearrange("p (t n) -> p t n", n=128),
                            pr[:].rearrange("p (t n) -> p t n", n=128),
                            qmv, ALU.add)
                        # wpre = xkg + wq1 ; wrel = max(wpre, 0)
                        wpre = wpp.tile([128, 512], BF16, name="wpre")
                        nc.vector.tensor_tensor(wpre[:], gv(0, q), wq1[:], ALU.add)
                        wrel = wrp.tile([128, 512], BF16, name="wrel")
                        nc.vector.tensor_scalar(wrel[:], wpre[:], 0.0, None, ALU.max)
                        pW1 = ps2.tile([128, 512], F32, name="pW1", tag="psq")
                        mm(pW1[0:CS, :], w_sb["lw1wb"][:], wrel[:])
                        w1r = w1p.tile([CS, 512], BF16, name="w1r")
                        nc.scalar.activation(w1r[:], pW1[0:CS, :], AF.Relu,
                                             bias=w_sb["w1be"][:])
                        pE = ps3.tile([128, 512], F32, name="pE", tag="psk")
                        mm(pE[:], w_sb["lw2repb"][:], w1r[:])
                        nc.scalar.activation(
                            ER3[:, qt, :].rearrange("p t n -> p (t n)"),
                            pE[:], AF.Exp, bias=w_sb["lw2brep"][:])
                        # V = xvg + pr ; VW = V * E
                        V = vp.tile([128, 512], BF16, name="V")
                        nc.vector.tensor_tensor(V[:], gv(1, q), pr[:], ALU.add)
                        nc.vector.tensor_tensor(
                            VW3[:, qt, :].rearrange("p t n -> p (t n)"),
                            V[:], ER3[:, qt, :].rearrange("p t n -> p (t n)"),
                            ALU.mult)
                    # tree-fold reductions over t (bf16 2x adds)
                    Zt = sp.tile([128, 128], F32, name="Zt", tag="Zt")
                    yt = sp.tile([128, 128], F32, name="yt", tag="yt")
                    for src, dst in ((ER3, Zt), (VW3, yt)):
                        f1 = fp.tile([128, 8, 128], BF16, name="f1", tag="f1")
                        nc.vector.tensor_tensor(f1[:], src[:, 0:8, :], src[:, 8:16, :], ALU.add)
                        f2 = fp.tile([128, 4, 128], BF16, name="f2", tag="f2")
                        nc.vector.tensor_tensor(f2[:], f1[:, 0:4, :], f1[:, 4:8, :], ALU.add)
                        f3 = fp.tile([128, 2, 128], BF16, name="f3", tag="f3")
                        nc.vector.tensor_tensor(f3[:], f2[:, 0:2, :], f2[:, 2:4, :], ALU.add)
                        nc.vector.tensor_tensor(dst[:], f3[:, 0, :], f3[:, 1, :], ALU.add)
                    R = sp.tile([128, 128], F32, name="R", tag="R")
                    nc.vector.reciprocal(R[:], Zt[:])
                    yn = sp.tile([128, 128], F32, name="yn", tag="yn")
                    nc.vector.tensor_tensor(yn[:], yt[:], R[:], ALU.mult)
                    nc.scalar.activation(y2b[:, slt], yn[:], AF.Relu, bias=w_sb["bn2b"][:])

                # ---- phase D: epilogue (out = x3; host adds p3) ----
                for c0 in (range(0, N, 512) if KPH == "full" else []):
                    sl = bass.ts(c0 // 512, 512)
                    pl = ps1.tile([128, 512], F32, name="pl3", tag="psA")
                    mm(pl[:], w_sb["lin3wb"][:], y2b[:, sl])
                    zf = op.tile([128, 512], F32, name="zf", tag="o")
                    nc.vector.tensor_tensor(zf[:], pl[:], tfb_sb[:, sl], ALU.add)
                    zb = zp.tile([128, 512], BF16, name="zb", tag="zb")
                    nc.scalar.activation(zb[:], zf[:], AF.Relu, bias=w_sb["bn3b"][:])
                    pm = ps2.tile([128, 512], F32, name="pm1", tag="psq")
                    mm(pm[0:64, :], w_sb["m1wb"][:], zb[:])
                    h2b = zp.tile([64, 512], BF16, name="h2b", tag="h2b")
                    nc.scalar.activation(h2b[:], pm[0:64, :], AF.Relu,
                                         bias=w_sb["m1be"][:])
                    pm2 = ps3.tile([128, 512], F32, name="pm2", tag="psk")
                    mm(pm2[0:3, :], w_sb["m2wb"][:], h2b[:])
                    ob = op.tile([3, 512], F32, name="ob", tag="ob")
                    nc.scalar.activation(ob[:], pm2[0:3, :], AF.Copy)
                    nc.sync.dma_start(out_d.ap()[:, sl], ob[:])

    nc.compile()
    return nc


def kernel(**inputs):
    f32 = lambda k: np.asarray(inputs[k], np.float32)
    pxo = f32("pxo")                       # [B,N,3]
    tf = f32("transf_features")            # [B,C,N]
    bf = lambda a: np.ascontiguousarray(np.asarray(a, np.float32)).astype(ml_dtypes.bfloat16)
    col = lambda a: np.ascontiguousarray(np.asarray(a, np.float32).reshape(-1, 1))

    b3 = f32("lp1b") + f32("lpbb")
    shared = {
        "lin1wb": bf(f32("lin1w")),
        "lp1wb": bf(np.pad(f32("lp1w"), ((0, 0), (0, 13)))),
        "wqb": bf(f32("wq")), "wkb": bf(f32("wk")), "wvb": bf(f32("wv")),
        "lp2wb": bf(f32("lp2w")), "lw1wb": bf(f32("lw1w")),
        "lw2repb": bf(np.tile(f32("lw2w"), (1, 8))),
        "lin3wb": bf(f32("lin3w")),
        "m1wb": bf(f32("m1w")), "m2wb": bf(f32("m2w")),
        "ident": bf(np.eye(128, dtype=np.float32)),
        "bias1": col(f32("bn1b")),
        "qmbias": col(f32("lwb1b") - f32("bq")),
        "bk": col(f32("bk")), "bv": col(f32("bv")),
        "b3": col(b3),
        "lp2b": col(f32("lp2b")),
        "w1be": col(f32("lw1b") + f32("lwb2b")),
        "lw2brep": col(np.tile(f32("lw2b"), 8)),
        "bn2b": col(f32("bn2b")), "bn3b": col(f32("bn3b")),
        "m1be": col(f32("m1b") + f32("mbb")),
    }

    in_maps = []
    for b in range(B):
        p = pxo[b]                                        # [N,3]
        sq = (p * p).sum(1)
        dmat = sq[:, None] + sq[None, :] - 2.0 * (p @ p.T)
        idx = np.argpartition(dmat, NS, axis=1)[:, :NS]   # [N,16] smallest set
        iw = np.empty((128, N), np.int16)
        for it in range(NT):
            L = idx[it * 128:(it + 1) * 128, :].T.reshape(2048)   # t-major pairs
            blk = L.reshape(128, 16).T.astype(np.int16)   # [16,128] wrapped
            iw[:, it * 128:(it + 1) * 128] = np.tile(blk, (8, 1))
        m = dict(shared)
        m["tfb"] = bf(tf[b])
        m["p3b"] = bf(p.T)
        m["iw"] = iw
        in_maps.append(m)

    _CACHE["in_maps"] = in_maps
    if "nc" not in _CACHE:
        _CACHE["nc"] = _build_nc()
    res = run_bass_kernel_spmd(_CACHE["nc"], in_maps, core_ids=list(range(8)))
    x3 = np.stack([np.asarray(res.results[i]["out"], np.float32)
                   for i in range(B)], axis=0)              # [B,3,N]
    return x3 + np.transpose(pxo, (0, 2, 1))
